# revision 14
# baseline (speedup 1.0000x reference)
"""Bass/Tile TRN2 kernel for nn_AttentionLayer (B=2, S=2048, D=1024, H=16).

Sharding: 8 cores = 2 (batch) x 4 (head groups of 4 heads each).
Each core computes Q/K/V projections for its 256 output columns and
full attention for its 4 heads; host concatenates the per-core
[S, 256] output slices.

Device-side layout choices:
  - Host pre-transposes q/k/v to x^T [D, S] so projections contract D on
    the partition dim with no on-device transposes.
  - Q^T, K^T produced head-transposed [e, s]; V produced natural [s, e]
    with a fused all-ones column per head (denominator rides the PV
    matmul as output row 64).
  - scores^T = K Q^T per head; softmax exp on ScalarE from PSUM (scale
    1/8 fused); no max-subtraction (scores are O(10), fp32 exp safe).
  - PV: out^T[h d+1, sq] = V'^T E accumulated over sk chunks in PSUM.
  - PE transpose of out^T -> out, then normalize by the ones-row sum.
  - All matmuls in float32r (TF32-like, 1 cycle/row at N>=256).
"""

import sys

sys.path.insert(0, "/opt/trn_rl_repo")

import numpy as np

import concourse.bacc as bacc
import concourse.mybir as mybir
from concourse.masks import make_identity
from concourse.tile import TileContext
from concourse.bass_utils import run_bass_kernel_spmd

F32 = mybir.dt.float32
F32R = mybir.dt.float32r
AF = mybir.ActivationFunctionType
ALU = mybir.AluOpType

B, S, D, H = 2, 2048, 1024, 16
HD = D // H            # 64
NCORES = 8
HPC = 4                # heads per core
E = HPC * HD           # 256 output cols per core
EV = HPC * (HD + 1)    # 260: V' with ones column per head
DCH = D // 128         # 8 d chunks
ST = S // 512          # 4 s tiles (projections)
SQT = S // 1024        # 2 sq tiles (attention)
SKC = S // 128         # 16 sk chunks
SCALE = 1.0 / np.sqrt(HD)


def build_kernel(repeat: int = 1, loop_n: int = 1):
    nc = bacc.Bacc()
    xqT = nc.dram_tensor("xqT", [D, S], F32, kind="ExternalInput")
    xkT = nc.dram_tensor("xkT", [D, S], F32, kind="ExternalInput")
    xvT = nc.dram_tensor("xvT", [D, S], F32, kind="ExternalInput")
    wq = nc.dram_tensor("wq", [D, E], F32, kind="ExternalInput")
    wk = nc.dram_tensor("wk", [D, E], F32, kind="ExternalInput")
    wv = nc.dram_tensor("wv", [D, EV], F32, kind="ExternalInput")
    bq = nc.dram_tensor("bq", [128, 2], F32, kind="ExternalInput")
    bk = nc.dram_tensor("bk", [128, 2], F32, kind="ExternalInput")
    bv = nc.dram_tensor("bv", [128, EV], F32, kind="ExternalInput")
    out = nc.dram_tensor("out", [S, E], F32, kind="ExternalOutput")

    with TileContext(nc) as tc:
        with tc.tile_pool(name="wsb", bufs=1) as wsb, \
             tc.tile_pool(name="xsb", bufs=3) as xsb, \
             tc.tile_pool(name="qkv", bufs=1) as qkv, \
             tc.tile_pool(name="esb", bufs=4) as esb, \
             tc.tile_pool(name="osb", bufs=4) as osb, \
             tc.tile_pool(name="pps", bufs=2, space="PSUM") as pps, \
             tc.tile_pool(name="stp", bufs=2, space="PSUM") as stp, \
             tc.tile_pool(name="pvp", bufs=2, space="PSUM") as pvp:

            # ---- constants / weights ----
            # load order matters at startup: wk then xk0 feed the first
            # matmuls; everything else trails behind them.
            wk_t = wsb.tile([128, DCH, E], F32R)
            nc.gpsimd.dma_start(wk_t[:], wk.rearrange("(c p) e -> p c e", p=128))
            bk_t = wsb.tile([128, 2], F32)
            nc.sync.dma_start(bk_t[:], bk[:])
            ident = wsb.tile([65, 65], F32)
            make_identity(nc, ident[:])
            # touch Exp early so the ACT table load happens during projections
            warm = wsb.tile([128, 1], F32)
            nc.scalar.activation(warm[:], bk_t[:, 0:1], AF.Exp)

            def load_wq():
                wq_t = wsb.tile([128, DCH, E], F32R, name="wq_t")
                nc.gpsimd.dma_start(wq_t[:], wq.rearrange("(c p) e -> p c e", p=128))
                bq_t = wsb.tile([128, 2], F32, name="bq_t")
                nc.sync.dma_start(bq_t[:], bq[:])
                return wq_t, bq_t

            def load_wv():
                wv_t = wsb.tile([128, DCH, EV], F32R, name="wv_t")
                nc.gpsimd.dma_start(wv_t[:], wv.rearrange("(c p) e -> p c e", p=128))
                bv_t = wsb.tile([128, EV], F32, name="bv_t")
                nc.sync.dma_start(bv_t[:], bv[:])
                return wv_t, bv_t

            def load_x(src, si):
                sl = slice(512 * si, 512 * (si + 1))
                x_t = xsb.tile([128, DCH, 512], F32R, tag="x", name=f"x_{si}")
                nc.gpsimd.dma_start(
                    x_t[:], src[:, sl].rearrange("(c p) s -> p c s", p=128))
                return x_t

            def project_qk(x_t, w_t, b_t, o_t, si, ets=(0, 1)):
                sl = slice(512 * si, 512 * (si + 1))
                for et in ets:
                    ps = pps.tile([128, 512], F32, tag="pj", name="ps_qk")
                    for c in range(DCH):
                        nc.tensor.matmul(
                            ps[:], w_t[:, c, 128 * et:128 * (et + 1)],
                            x_t[:, c], start=(c == 0), stop=(c == DCH - 1))
                    nc.vector.tensor_scalar(
                        out=o_t[:, et, sl], in0=ps[:],
                        scalar1=b_t[:, et:et + 1], scalar2=None, op0=ALU.add)

            def project_v(x_t, wv_t, bv_t, si):
                for k in range(4):
                    psv = pps.tile([128, EV], F32, tag="pj", name="ps_v")
                    for c in range(DCH):
                        nc.tensor.matmul(
                            psv[:], x_t[:, c, 128 * k:128 * (k + 1)],
                            wv_t[:, c], start=(c == 0), stop=(c == DCH - 1))
                    nc.vector.tensor_tensor(
                        out=V_t[:, 4 * si + k, :], in0=psv[:], in1=bv_t[:],
                        op=ALU.add)

            def attention_piece(ov_acc, pr, sqt, si):
                """Chunks 4si..4si+3 of the (pr, sqt) block.

                PV partials land in a transient PSUM tile per chunk pair and
                are accumulated into ov_acc[h] (SBUF) on the DVE, so only one
                pv PSUM slot is held at a time and many blocks can be in
                flight chunk-chasing the K/V loads.
                """
                sq0 = 512 * sqt
                pv_a = pvp.tile([65, 512], F32, tag="pv", name="pv_a")
                pv_b = pvp.tile([65, 512], F32, tag="pv", name="pv_b")
                pvs = (pv_a, pv_b)
                for cpl in range(2):           # chunk pairs within the piece
                    sts = []
                    for h in range(2):
                        st = stp.tile([128, 1024], F32, tag="st", name=f"st{h}")
                        sts.append(st)
                    # interleave the two heads' QK matmuls: they use disjoint
                    # PE row groups (hd 0-63 vs 64-127) and can run
                    # concurrently when adjacent in the PE queue
                    for q in range(2):
                        for h in range(2):
                            hp = slice(64 * h, 64 * (h + 1))
                            ck = 4 * si + 2 * cpl + q
                            nc.tensor.matmul(
                                sts[h][:, 512 * q:512 * (q + 1)],
                                KT_t[hp, pr, 128 * ck:128 * (ck + 1)],
                                QT_t[hp, pr, sq0:sq0 + 512],
                                start=True, stop=True)
                    for h in range(2):
                        hh = 2 * pr + h
                        e_t = esb.tile([128, 1024], F32R, name="e_t")
                        nc.scalar.activation(e_t[:], sts[h][:], AF.Exp,
                                             scale=float(SCALE))
                        for q in range(2):
                            ck = 4 * si + 2 * cpl + q
                            nc.tensor.matmul(
                                pvs[h][:],
                                V_t[:, ck, 65 * hh:65 * hh + 65],
                                e_t[:, 512 * q:512 * (q + 1)],
                                start=(cpl == 0 and q == 0),
                                stop=(cpl == 1 and q == 1))
                for h in range(2):
                    if si == 0:
                        nc.vector.tensor_copy(ov_acc[h][:], pvs[h][:])
                    else:
                        nc.vector.tensor_tensor(
                            out=ov_acc[h][:], in0=ov_acc[h][:], in1=pvs[h][:],
                            op=ALU.add)

            def drain_head(ov_acc, pr, sqt, h):
                sq0 = 512 * sqt
                if True:
                    hh = 2 * pr + h
                    for k in range(4):
                        ot = pps.tile([128, 65], F32, tag="pj", name="ot")
                        nc.tensor.transpose(
                            ot[:], ov_acc[h][:, 128 * k:128 * (k + 1)], ident[:])
                        rc = osb.tile([128, 1], F32, tag="rc", name="rc")
                        nc.vector.reciprocal(rc[:], ot[:, 64:65])
                        ob = osb.tile([128, HD], F32, tag="ob", name="ob")
                        nc.vector.tensor_scalar(
                            out=ob[:], in0=ot[:, 0:HD], scalar1=rc[:],
                            scalar2=None, op0=ALU.mult)
                        r0 = sq0 + 128 * k
                        nc.sync.dma_start(
                            out[r0:r0 + 128, HD * hh:HD * (hh + 1)], ob[:])

            def attention_drain(ov_acc, pr, sqt):
                """Transpose + normalize + store the (pr, sqt) block."""
                for h in range(2):
                    drain_head(ov_acc, pr, sqt, h)

            def new_block(pr, sqt):
                a = osb.tile([65, 512], F32, tag="ov", bufs=12, name=f"ova{pr}{sqt}")
                b = osb.tile([65, 512], F32, tag="ov", bufs=12, name=f"ovb{pr}{sqt}")
                return (a, b)

            import contextlib

            def body_scope():
                if loop_n > 1:
                    return tc.For_i(0, loop_n, 1)
                return contextlib.nullcontext()

            for _ in range(repeat):
              with body_scope():
                # persistent per-iteration products
                QT_t = qkv.tile([128, 2, S], F32R, tag="QT", name="QT_t")
                KT_t = qkv.tile([128, 2, S], F32R, tag="KT", name="KT_t")
                V_t = qkv.tile([128, SKC, EV], F32R, tag="V", name="V_t")

                ov = {}

                def pieces(*keys):
                    for (pr, sqt, si) in keys:
                        if (pr, sqt) not in ov:
                            ov[(pr, sqt)] = new_block(pr, sqt)
                        attention_piece(ov[(pr, sqt)], pr, sqt, si)

                # si=0 data first, then attention pieces chunk-chase the
                # remaining K/V (+Q) loads; drains are deferred into later
                # blocks' ACT-busy windows.
                xk = load_x(xkT, 0)
                project_qk(xk, wk_t, bk_t, KT_t, 0)
                wq_t, bq_t = load_wq()
                xq = load_x(xqT, 0)
                project_qk(xq, wq_t, bq_t, QT_t, 0)
                wv_t, bv_t = load_wv()
                xv = load_x(xvT, 0)
                project_v(xv, wv_t, bv_t, 0)
                pieces((0, 0, 0), (1, 0, 0))

                xk = load_x(xkT, 1)
                project_qk(xk, wk_t, bk_t, KT_t, 1)
                xv = load_x(xvT, 1)
                project_v(xv, wv_t, bv_t, 1)
                pieces((0, 0, 1), (1, 0, 1))
                xq = load_x(xqT, 1)
                project_qk(xq, wq_t, bq_t, QT_t, 1)
                pieces((0, 1, 0), (1, 1, 0))

                xk = load_x(xkT, 2)
                project_qk(xk, wk_t, bk_t, KT_t, 2)
                xv = load_x(xvT, 2)
                project_v(xv, wv_t, bv_t, 2)
                pieces((0, 0, 2), (1, 0, 2), (0, 1, 1), (1, 1, 1))

                xk = load_x(xkT, 3)
                project_qk(xk, wk_t, bk_t, KT_t, 3)
                xv = load_x(xvT, 3)
                project_v(xv, wv_t, bv_t, 3)
                pieces((0, 0, 3), (1, 0, 3))        # sqt0 complete

                xq = load_x(xqT, 2)
                project_qk(xq, wq_t, bq_t, QT_t, 2)
                pieces((0, 1, 2), (1, 1, 2))
                attention_drain(ov[(0, 0)], 0, 0)
                pieces((0, 1, 3), (1, 1, 3))        # sqt1 complete
                attention_drain(ov[(1, 0)], 1, 0)

                xq = load_x(xqT, 3)
                project_qk(xq, wq_t, bq_t, QT_t, 3)
                pieces((0, 2, 0), (1, 2, 0))
                attention_drain(ov[(0, 1)], 0, 1)
                pieces((0, 2, 1), (1, 2, 1))
                attention_drain(ov[(1, 1)], 1, 1)
                pieces((0, 2, 2), (1, 2, 2), (0, 2, 3), (1, 2, 3))

                pieces((0, 3, 0), (1, 3, 0))
                attention_drain(ov[(0, 2)], 0, 2)
                pieces((0, 3, 1), (1, 3, 1))
                attention_drain(ov[(1, 2)], 1, 2)
                pieces((0, 3, 2), (1, 3, 2), (0, 3, 3), (1, 3, 3))
                attention_drain(ov[(0, 3)], 0, 3)
                attention_drain(ov[(1, 3)], 1, 3)
    nc.compile()
    return nc


_NC_CACHE = {}


def _get_nc(repeat: int = 1, loop_n: int = 1):
    key = (repeat, loop_n)
    if key not in _NC_CACHE:
        _NC_CACHE[key] = build_kernel(repeat, loop_n)
    return _NC_CACHE[key]


def _shard_inputs(q, k, v, Wq, bq, Wk, bk, Wv, bv):
    """Build the 8 per-core input maps (host-side marshaling)."""
    xT = {}
    for b in range(B):
        xT[("q", b)] = np.ascontiguousarray(np.asarray(q)[b].T)
        xT[("k", b)] = np.ascontiguousarray(np.asarray(k)[b].T)
        xT[("v", b)] = np.ascontiguousarray(np.asarray(v)[b].T)
    Wq, Wk, Wv = (np.asarray(a, np.float32) for a in (Wq, Wk, Wv))
    bq, bk, bv = (np.asarray(a, np.float32) for a in (bq, bk, bv))
    in_maps = []
    for c in range(NCORES):
        b, g = divmod(c, HPC)
        sl = slice(E * g, E * (g + 1))
        wv_p = np.zeros((D, EV), np.float32)
        bv_p = np.zeros((128, EV), np.float32)
        for h in range(HPC):
            wv_p[:, 65 * h:65 * h + HD] = Wv[:, E * g + HD * h:E * g + HD * (h + 1)]
            bv_p[:, 65 * h:65 * h + HD] = bv[E * g + HD * h:E * g + HD * (h + 1)]
            bv_p[:, 65 * h + HD] = 1.0
        in_maps.append({
            "xqT": xT[("q", b)], "xkT": xT[("k", b)], "xvT": xT[("v", b)],
            "wq": np.ascontiguousarray(Wq[:, sl]),
            "wk": np.ascontiguousarray(Wk[:, sl]),
            "wv": wv_p,
            "bq": np.ascontiguousarray(bq[sl].reshape(2, 128).T),
            "bk": np.ascontiguousarray(bk[sl].reshape(2, 128).T),
            "bv": bv_p,
        })
    return in_maps


def kernel(q, k, v, Wq, bq, Wk, bk, Wv, bv):
    nc = _get_nc()
    in_maps = _shard_inputs(q, k, v, Wq, bq, Wk, bk, Wv, bv)
    res = run_bass_kernel_spmd(nc, in_maps, core_ids=list(range(NCORES)))
    outp = np.empty((B, S, D), np.float32)
    for c in range(NCORES):
        b, g = divmod(c, HPC)
        outp[b, :, E * g:E * (g + 1)] = res.results[c]["out"]
    return outp


# revision 15
# speedup vs baseline: 3.4511x; 3.4511x over previous
"""Bass/Tile TRN2 kernel for nn_AttentionLayer (B=2, S=2048, D=1024, H=16).

Sharding: 8 cores = 2 (batch) x 4 (head groups of 4 heads each).
Each core computes Q/K/V projections for its 256 output columns and
full attention for its 4 heads; host concatenates the per-core
[S, 256] output slices.

Device-side layout choices:
  - Host pre-transposes q/k/v to x^T [D, S] so projections contract D on
    the partition dim with no on-device transposes.
  - Q^T, K^T produced head-transposed [e, s]; V produced natural [s, e]
    with a fused all-ones column per head (denominator rides the PV
    matmul as output row 64).
  - scores^T = K Q^T per head; softmax exp on ScalarE from PSUM (scale
    1/8 fused); no max-subtraction (scores are O(10), fp32 exp safe).
  - PV: out^T[h d+1, sq] = V'^T E accumulated over sk chunks in PSUM.
  - PE transpose of out^T -> out, then normalize by the ones-row sum.
  - All matmuls in float32r (TF32-like, 1 cycle/row at N>=256).
"""

import sys

sys.path.insert(0, "/opt/trn_rl_repo")

import numpy as np

import concourse.bacc as bacc
import concourse.mybir as mybir
from concourse.masks import make_identity
from concourse.tile import TileContext
from concourse.bass_utils import run_bass_kernel_spmd

F32 = mybir.dt.float32
F32R = mybir.dt.float32r
AF = mybir.ActivationFunctionType
ALU = mybir.AluOpType

B, S, D, H = 2, 2048, 1024, 16
HD = D // H            # 64
NCORES = 8
HPC = 4                # heads per core
E = HPC * HD           # 256 output cols per core
EV = HPC * (HD + 1)    # 260: V' with ones column per head
DCH = D // 128         # 8 d chunks
ST = S // 512          # 4 s tiles (projections)
SQT = S // 1024        # 2 sq tiles (attention)
SKC = S // 128         # 16 sk chunks
SCALE = 1.0 / np.sqrt(HD)


def build_kernel(repeat: int = 1, loop_n: int = 1):
    nc = bacc.Bacc()
    xqT = nc.dram_tensor("xqT", [D, S], F32R, kind="ExternalInput")
    xkT = nc.dram_tensor("xkT", [D, S], F32R, kind="ExternalInput")
    xvT = nc.dram_tensor("xvT", [D, S], F32R, kind="ExternalInput")
    wq = nc.dram_tensor("wq", [D, E], F32R, kind="ExternalInput")
    wk = nc.dram_tensor("wk", [D, E], F32R, kind="ExternalInput")
    wv = nc.dram_tensor("wv", [D, EV], F32R, kind="ExternalInput")
    bq = nc.dram_tensor("bq", [128, 2], F32, kind="ExternalInput")
    bk = nc.dram_tensor("bk", [128, 2], F32, kind="ExternalInput")
    bv = nc.dram_tensor("bv", [128, EV], F32, kind="ExternalInput")
    out = nc.dram_tensor("out", [S, E], F32, kind="ExternalOutput")

    with TileContext(nc) as tc:
        with tc.tile_pool(name="wsb", bufs=1) as wsb, \
             tc.tile_pool(name="xsb", bufs=3) as xsb, \
             tc.tile_pool(name="qkv", bufs=1) as qkv, \
             tc.tile_pool(name="esb", bufs=4) as esb, \
             tc.tile_pool(name="osb", bufs=4) as osb, \
             tc.tile_pool(name="pps", bufs=2, space="PSUM") as pps, \
             tc.tile_pool(name="stp", bufs=2, space="PSUM") as stp, \
             tc.tile_pool(name="pvp", bufs=2, space="PSUM") as pvp:

            # ---- constants / weights ----
            # load order matters at startup: wk then xk0 feed the first
            # matmuls; everything else trails behind them.
            wk_t = wsb.tile([128, DCH, E], F32R)
            nc.sync.dma_start(wk_t[:], wk.rearrange("(c p) e -> p c e", p=128))
            bk_t = wsb.tile([128, 2], F32)
            nc.sync.dma_start(bk_t[:], bk[:])
            ident = wsb.tile([65, 65], F32)
            make_identity(nc, ident[:])
            # touch Exp early so the ACT table load happens during projections
            warm = wsb.tile([128, 1], F32)
            nc.scalar.activation(warm[:], bk_t[:, 0:1], AF.Exp)

            def load_wq():
                wq_t = wsb.tile([128, DCH, E], F32R, name="wq_t")
                nc.sync.dma_start(wq_t[:], wq.rearrange("(c p) e -> p c e", p=128))
                bq_t = wsb.tile([128, 2], F32, name="bq_t")
                nc.sync.dma_start(bq_t[:], bq[:])
                return wq_t, bq_t

            def load_wv():
                wv_t = wsb.tile([128, DCH, EV], F32R, name="wv_t")
                nc.sync.dma_start(wv_t[:], wv.rearrange("(c p) e -> p c e", p=128))
                bv_t = wsb.tile([128, EV], F32, name="bv_t")
                nc.sync.dma_start(bv_t[:], bv[:])
                return wv_t, bv_t

            def load_x(src, si):
                sl = slice(512 * si, 512 * (si + 1))
                x_t = xsb.tile([128, DCH, 512], F32R, tag="x", name=f"x_{si}")
                nc.sync.dma_start(
                    x_t[:], src[:, sl].rearrange("(c p) s -> p c s", p=128))
                return x_t

            def project_qk(x_t, w_t, b_t, o_t, si, ets=(0, 1)):
                sl = slice(512 * si, 512 * (si + 1))
                for et in ets:
                    ps = pps.tile([128, 512], F32, tag="pj", name="ps_qk")
                    for c in range(DCH):
                        nc.tensor.matmul(
                            ps[:], w_t[:, c, 128 * et:128 * (et + 1)],
                            x_t[:, c], start=(c == 0), stop=(c == DCH - 1))
                    nc.vector.tensor_scalar(
                        out=o_t[:, et, sl], in0=ps[:],
                        scalar1=b_t[:, et:et + 1], scalar2=None, op0=ALU.add)

            def project_v(x_t, wv_t, bv_t, si):
                for k in range(4):
                    psv = pps.tile([128, EV], F32, tag="pj", name="ps_v")
                    for c in range(DCH):
                        nc.tensor.matmul(
                            psv[:], x_t[:, c, 128 * k:128 * (k + 1)],
                            wv_t[:, c], start=(c == 0), stop=(c == DCH - 1))
                    nc.vector.tensor_tensor(
                        out=V_t[:, 4 * si + k, :], in0=psv[:], in1=bv_t[:],
                        op=ALU.add)

            def attention_piece(ov_acc, pr, sqt, si):
                """Chunks 4si..4si+3 of the (pr, sqt) block.

                PV partials land in a transient PSUM tile per chunk pair and
                are accumulated into ov_acc[h] (SBUF) on the DVE, so only one
                pv PSUM slot is held at a time and many blocks can be in
                flight chunk-chasing the K/V loads.
                """
                sq0 = 512 * sqt
                pv_a = pvp.tile([65, 512], F32, tag="pv", name="pv_a")
                pv_b = pvp.tile([65, 512], F32, tag="pv", name="pv_b")
                pvs = (pv_a, pv_b)
                for cpl in range(2):           # chunk pairs within the piece
                    sts = []
                    for h in range(2):
                        st = stp.tile([128, 1024], F32, tag="st", name=f"st{h}")
                        sts.append(st)
                    # interleave the two heads' QK matmuls: they use disjoint
                    # PE row groups (hd 0-63 vs 64-127) and can run
                    # concurrently when adjacent in the PE queue
                    for q in range(2):
                        for h in range(2):
                            hp = slice(64 * h, 64 * (h + 1))
                            ck = 4 * si + 2 * cpl + q
                            nc.tensor.matmul(
                                sts[h][:, 512 * q:512 * (q + 1)],
                                KT_t[hp, pr, 128 * ck:128 * (ck + 1)],
                                QT_t[hp, pr, sq0:sq0 + 512],
                                start=True, stop=True)
                    for h in range(2):
                        hh = 2 * pr + h
                        e_t = esb.tile([128, 1024], F32R, name="e_t")
                        nc.scalar.activation(e_t[:], sts[h][:], AF.Exp,
                                             scale=float(SCALE))
                        for q in range(2):
                            ck = 4 * si + 2 * cpl + q
                            nc.tensor.matmul(
                                pvs[h][:],
                                V_t[:, ck, 65 * hh:65 * hh + 65],
                                e_t[:, 512 * q:512 * (q + 1)],
                                start=(cpl == 0 and q == 0),
                                stop=(cpl == 1 and q == 1))
                for h in range(2):
                    if si == 0:
                        nc.vector.tensor_copy(ov_acc[h][:], pvs[h][:])
                    else:
                        nc.vector.tensor_tensor(
                            out=ov_acc[h][:], in0=ov_acc[h][:], in1=pvs[h][:],
                            op=ALU.add)

            def drain_head(ov_acc, pr, sqt, h):
                sq0 = 512 * sqt
                if True:
                    hh = 2 * pr + h
                    for k in range(4):
                        ot = pps.tile([128, 65], F32, tag="pj", name="ot")
                        nc.tensor.transpose(
                            ot[:], ov_acc[h][:, 128 * k:128 * (k + 1)], ident[:])
                        rc = osb.tile([128, 1], F32, tag="rc", name="rc")
                        nc.vector.reciprocal(rc[:], ot[:, 64:65])
                        ob = osb.tile([128, HD], F32, tag="ob", name="ob")
                        nc.vector.tensor_scalar(
                            out=ob[:], in0=ot[:, 0:HD], scalar1=rc[:],
                            scalar2=None, op0=ALU.mult)
                        r0 = sq0 + 128 * k
                        nc.sync.dma_start(
                            out[r0:r0 + 128, HD * hh:HD * (hh + 1)], ob[:])

            def attention_drain(ov_acc, pr, sqt):
                """Transpose + normalize + store the (pr, sqt) block."""
                for h in range(2):
                    drain_head(ov_acc, pr, sqt, h)

            def new_block(pr, sqt):
                a = osb.tile([65, 512], F32, tag="ov", bufs=12, name=f"ova{pr}{sqt}")
                b = osb.tile([65, 512], F32, tag="ov", bufs=12, name=f"ovb{pr}{sqt}")
                return (a, b)

            import contextlib

            def body_scope():
                if loop_n > 1:
                    return tc.For_i(0, loop_n, 1)
                return contextlib.nullcontext()

            for _ in range(repeat):
              with body_scope():
                # persistent per-iteration products
                QT_t = qkv.tile([128, 2, S], F32R, tag="QT", name="QT_t")
                KT_t = qkv.tile([128, 2, S], F32R, tag="KT", name="KT_t")
                V_t = qkv.tile([128, SKC, EV], F32R, tag="V", name="V_t")

                ov = {}

                def pieces(*keys):
                    for (pr, sqt, si) in keys:
                        if (pr, sqt) not in ov:
                            ov[(pr, sqt)] = new_block(pr, sqt)
                        attention_piece(ov[(pr, sqt)], pr, sqt, si)

                # si=0 data first, then attention pieces chunk-chase the
                # remaining K/V (+Q) loads; drains are deferred into later
                # blocks' ACT-busy windows.
                xk = load_x(xkT, 0)
                project_qk(xk, wk_t, bk_t, KT_t, 0)
                wq_t, bq_t = load_wq()
                xq = load_x(xqT, 0)
                project_qk(xq, wq_t, bq_t, QT_t, 0)
                wv_t, bv_t = load_wv()
                xv = load_x(xvT, 0)
                project_v(xv, wv_t, bv_t, 0)
                pieces((0, 0, 0), (1, 0, 0))

                xk = load_x(xkT, 1)
                project_qk(xk, wk_t, bk_t, KT_t, 1)
                xv = load_x(xvT, 1)
                project_v(xv, wv_t, bv_t, 1)
                pieces((0, 0, 1), (1, 0, 1))
                xq = load_x(xqT, 1)
                project_qk(xq, wq_t, bq_t, QT_t, 1)
                pieces((0, 1, 0), (1, 1, 0))

                xk = load_x(xkT, 2)
                project_qk(xk, wk_t, bk_t, KT_t, 2)
                xv = load_x(xvT, 2)
                project_v(xv, wv_t, bv_t, 2)
                pieces((0, 0, 2), (1, 0, 2), (0, 1, 1), (1, 1, 1))

                xk = load_x(xkT, 3)
                project_qk(xk, wk_t, bk_t, KT_t, 3)
                xv = load_x(xvT, 3)
                project_v(xv, wv_t, bv_t, 3)
                pieces((0, 0, 3), (1, 0, 3))        # sqt0 complete

                xq = load_x(xqT, 2)
                project_qk(xq, wq_t, bq_t, QT_t, 2)
                pieces((0, 1, 2), (1, 1, 2))
                attention_drain(ov[(0, 0)], 0, 0)
                pieces((0, 1, 3), (1, 1, 3))        # sqt1 complete
                attention_drain(ov[(1, 0)], 1, 0)

                xq = load_x(xqT, 3)
                project_qk(xq, wq_t, bq_t, QT_t, 3)
                pieces((0, 2, 0), (1, 2, 0))
                attention_drain(ov[(0, 1)], 0, 1)
                pieces((0, 2, 1), (1, 2, 1))
                attention_drain(ov[(1, 1)], 1, 1)
                pieces((0, 2, 2), (1, 2, 2), (0, 2, 3), (1, 2, 3))

                pieces((0, 3, 0), (1, 3, 0))
                attention_drain(ov[(0, 2)], 0, 2)
                pieces((0, 3, 1), (1, 3, 1))
                attention_drain(ov[(1, 2)], 1, 2)
                pieces((0, 3, 2), (1, 3, 2), (0, 3, 3), (1, 3, 3))
                attention_drain(ov[(0, 3)], 0, 3)
                attention_drain(ov[(1, 3)], 1, 3)
    nc.compile()
    return nc


_NC_CACHE = {}


def _get_nc(repeat: int = 1, loop_n: int = 1):
    key = (repeat, loop_n)
    if key not in _NC_CACHE:
        _NC_CACHE[key] = build_kernel(repeat, loop_n)
    return _NC_CACHE[key]


def _shard_inputs(q, k, v, Wq, bq, Wk, bk, Wv, bv):
    """Build the 8 per-core input maps (host-side marshaling)."""
    xT = {}
    for b in range(B):
        xT[("q", b)] = np.ascontiguousarray(np.asarray(q)[b].T)
        xT[("k", b)] = np.ascontiguousarray(np.asarray(k)[b].T)
        xT[("v", b)] = np.ascontiguousarray(np.asarray(v)[b].T)
    Wq, Wk, Wv = (np.asarray(a, np.float32) for a in (Wq, Wk, Wv))
    bq, bk, bv = (np.asarray(a, np.float32) for a in (bq, bk, bv))
    in_maps = []
    for c in range(NCORES):
        b, g = divmod(c, HPC)
        sl = slice(E * g, E * (g + 1))
        wv_p = np.zeros((D, EV), np.float32)
        bv_p = np.zeros((128, EV), np.float32)
        for h in range(HPC):
            wv_p[:, 65 * h:65 * h + HD] = Wv[:, E * g + HD * h:E * g + HD * (h + 1)]
            bv_p[:, 65 * h:65 * h + HD] = bv[E * g + HD * h:E * g + HD * (h + 1)]
            bv_p[:, 65 * h + HD] = 1.0
        in_maps.append({
            "xqT": xT[("q", b)], "xkT": xT[("k", b)], "xvT": xT[("v", b)],
            "wq": np.ascontiguousarray(Wq[:, sl]),
            "wk": np.ascontiguousarray(Wk[:, sl]),
            "wv": wv_p,
            "bq": np.ascontiguousarray(bq[sl].reshape(2, 128).T),
            "bk": np.ascontiguousarray(bk[sl].reshape(2, 128).T),
            "bv": bv_p,
        })
    return in_maps


def kernel(q, k, v, Wq, bq, Wk, bk, Wv, bv):
    nc = _get_nc()
    in_maps = _shard_inputs(q, k, v, Wq, bq, Wk, bk, Wv, bv)
    res = run_bass_kernel_spmd(nc, in_maps, core_ids=list(range(NCORES)))
    outp = np.empty((B, S, D), np.float32)
    for c in range(NCORES):
        b, g = divmod(c, HPC)
        outp[b, :, E * g:E * (g + 1)] = res.results[c]["out"]
    return outp


# revision 19
# speedup vs baseline: 3.7445x; 1.0850x over previous
"""Bass/Tile TRN2 kernel for nn_AttentionLayer (B=2, S=2048, D=1024, H=16).

Sharding: 8 cores = 2 (batch) x 4 (head groups of 4 heads each).
Each core computes Q/K/V projections for its 256 output columns and
full attention for its 4 heads; host concatenates the per-core
[S, 256] output slices.

Device-side layout choices:
  - Host pre-transposes q/k/v to x^T [D, S] so projections contract D on
    the partition dim with no on-device transposes.
  - Q^T, K^T produced head-transposed [e, s]; V produced natural [s, e]
    with a fused all-ones column per head (denominator rides the PV
    matmul as output row 64).
  - scores^T = K Q^T per head; softmax exp on ScalarE from PSUM (scale
    1/8 fused); no max-subtraction (scores are O(10), fp32 exp safe).
  - PV: out^T[h d+1, sq] = V'^T E accumulated over sk chunks in PSUM.
  - PE transpose of out^T -> out, then normalize by the ones-row sum.
  - All matmuls in float32r (TF32-like, 1 cycle/row at N>=256).
"""

import sys

sys.path.insert(0, "/opt/trn_rl_repo")

import numpy as np

import concourse.bacc as bacc
import concourse.mybir as mybir
from concourse.masks import make_identity
from concourse.tile import TileContext
from concourse.bass_utils import run_bass_kernel_spmd

F32 = mybir.dt.float32
F32R = mybir.dt.float32r
AF = mybir.ActivationFunctionType
ALU = mybir.AluOpType

B, S, D, H = 2, 2048, 1024, 16
HD = D // H            # 64
NCORES = 8
HPC = 4                # heads per core
E = HPC * HD           # 256 output cols per core
EV = HPC * (HD + 1)    # 260: V' with ones column per head
DCH = D // 128         # 8 d chunks
ST = S // 512          # 4 s tiles (projections)
SQT = S // 1024        # 2 sq tiles (attention)
SKC = S // 128         # 16 sk chunks
SCALE = 1.0 / np.sqrt(HD)


def build_kernel(repeat: int = 1, loop_n: int = 1):
    nc = bacc.Bacc()
    xqT = nc.dram_tensor("xqT", [D, S], F32R, kind="ExternalInput")
    xkT = nc.dram_tensor("xkT", [D, S], F32R, kind="ExternalInput")
    xvT = nc.dram_tensor("xvT", [D, S], F32R, kind="ExternalInput")
    wq = nc.dram_tensor("wq", [D, E], F32R, kind="ExternalInput")
    wk = nc.dram_tensor("wk", [D, E], F32R, kind="ExternalInput")
    wv = nc.dram_tensor("wv", [D, EV], F32R, kind="ExternalInput")
    bq = nc.dram_tensor("bq", [128, 2], F32, kind="ExternalInput")
    bk = nc.dram_tensor("bk", [128, 2], F32, kind="ExternalInput")
    bv = nc.dram_tensor("bv", [128, EV], F32, kind="ExternalInput")
    out = nc.dram_tensor("out", [S, E], F32, kind="ExternalOutput")

    with TileContext(nc) as tc:
        with tc.tile_pool(name="wsb", bufs=1) as wsb, \
             tc.tile_pool(name="xsb", bufs=3) as xsb, \
             tc.tile_pool(name="qkv", bufs=1) as qkv, \
             tc.tile_pool(name="esb", bufs=4) as esb, \
             tc.tile_pool(name="osb", bufs=4) as osb, \
             tc.tile_pool(name="pps", bufs=1, space="PSUM") as pps, \
             tc.tile_pool(name="stp", bufs=2, space="PSUM") as stp, \
             tc.tile_pool(name="pvp", bufs=3, space="PSUM") as pvp:

            # ---- constants / weights ----
            # load order matters at startup: wk then xk0 feed the first
            # matmuls; everything else trails behind them.
            wk_t = wsb.tile([128, DCH, E], F32R)
            nc.sync.dma_start(wk_t[:], wk.rearrange("(c p) e -> p c e", p=128))
            bk_t = wsb.tile([128, 2], F32)
            nc.sync.dma_start(bk_t[:], bk[:])
            ident = wsb.tile([65, 65], F32)
            make_identity(nc, ident[:])
            # touch Exp early so the ACT table load happens during projections
            warm = wsb.tile([128, 1], F32)
            nc.scalar.activation(warm[:], bk_t[:, 0:1], AF.Exp)

            def load_wq():
                wq_t = wsb.tile([128, DCH, E], F32R, name="wq_t")
                nc.sync.dma_start(wq_t[:], wq.rearrange("(c p) e -> p c e", p=128))
                bq_t = wsb.tile([128, 2], F32, name="bq_t")
                nc.sync.dma_start(bq_t[:], bq[:])
                return wq_t, bq_t

            def load_wv():
                wv_t = wsb.tile([128, DCH, EV], F32R, name="wv_t")
                nc.sync.dma_start(wv_t[:], wv.rearrange("(c p) e -> p c e", p=128))
                bv_t = wsb.tile([128, EV], F32, name="bv_t")
                nc.sync.dma_start(bv_t[:], bv[:])
                return wv_t, bv_t

            def load_x(src, si):
                sl = slice(512 * si, 512 * (si + 1))
                x_t = xsb.tile([128, DCH, 512], F32R, tag="x", name=f"x_{si}")
                nc.sync.dma_start(
                    x_t[:], src[:, sl].rearrange("(c p) s -> p c s", p=128))
                return x_t

            def project_qk(x_t, w_t, b_t, o_t, si, kt=False):
                sl = slice(512 * si, 512 * (si + 1))
                for et in range(2):
                    ps = pps.tile([128, 512], F32, tag="pj", name="ps_qk")
                    for c in range(DCH):
                        nc.tensor.matmul(
                            ps[:], w_t[:, c, 128 * et:128 * (et + 1)],
                            x_t[:, c], start=(c == 0), stop=(c == DCH - 1))
                    if kt:
                        nc.vector.tensor_scalar(
                            out=o_t[0:64, 2 * et, sl], in0=ps[0:64, :],
                            scalar1=b_t[0:64, et:et + 1], scalar2=None,
                            op0=ALU.add)
                        nc.vector.tensor_scalar(
                            out=o_t[64:128, 2 * et + 1, sl], in0=ps[64:128, :],
                            scalar1=b_t[64:128, et:et + 1], scalar2=None,
                            op0=ALU.add)
                    else:
                        nc.vector.tensor_scalar(
                            out=o_t[:, et, sl], in0=ps[:],
                            scalar1=b_t[:, et:et + 1], scalar2=None,
                            op0=ALU.add)

            def project_v(x_t, wv_t, bv_t, si):
                for k in range(4):
                    psv = pps.tile([128, EV], F32, tag="pj", name="ps_v")
                    for c in range(DCH):
                        nc.tensor.matmul(
                            psv[:], x_t[:, c, 128 * k:128 * (k + 1)],
                            wv_t[:, c], start=(c == 0), stop=(c == DCH - 1))
                    nc.vector.tensor_tensor(
                        out=V_t[:, 4 * si + k, :], in0=psv[:], in1=bv_t[:],
                        op=ALU.add)

            def attention_piece(ov_acc, pr, sqt, si):
                """Chunks 4si..4si+3 of the (pr, sqt) block.

                PV partials land in a transient PSUM tile per chunk pair and
                are accumulated into ov_acc[h] (SBUF) on the DVE, so only one
                pv PSUM slot is held at a time and many blocks can be in
                flight chunk-chasing the K/V loads.
                """
                sq0 = 512 * sqt
                pv_a = pvp.tile([65, 512], F32, tag="pv", name="pv_a")
                pv_b = pvp.tile([65, 512], F32, tag="pv", name="pv_b")
                pvs = (pv_a, pv_b)
                for cpl in range(2):           # chunk pairs within the piece
                    sts = []
                    for h in range(2):
                        st = stp.tile([128, 1024], F32, tag="st", name=f"st{h}")
                        sts.append(st)
                    for q in range(2):
                        for h in range(2):
                            hh = 2 * pr + h
                            ck = 4 * si + 2 * cpl + q
                            nc.tensor.matmul(
                                sts[h][:, 512 * q:512 * (q + 1)],
                                KT_t[:, hh, 128 * ck:128 * (ck + 1)],
                                QT_t[:, pr, sq0:sq0 + 512],
                                start=True, stop=True)
                    for h in range(2):
                        hh = 2 * pr + h
                        e_t = esb.tile([128, 1024], F32R, name="e_t")
                        nc.scalar.activation(e_t[:], sts[h][:], AF.Exp,
                                             scale=float(SCALE))
                        for q in range(2):
                            ck = 4 * si + 2 * cpl + q
                            nc.tensor.matmul(
                                pvs[h][:],
                                V_t[:, ck, 65 * hh:65 * hh + 65],
                                e_t[:, 512 * q:512 * (q + 1)],
                                start=(cpl == 0 and q == 0),
                                stop=(cpl == 1 and q == 1))
                for h in range(2):
                    if si == 0:
                        nc.vector.tensor_copy(ov_acc[h][:], pvs[h][:])
                    else:
                        nc.vector.tensor_tensor(
                            out=ov_acc[h][:], in0=ov_acc[h][:], in1=pvs[h][:],
                            op=ALU.add)

            def drain_head(ov_acc, pr, sqt, h):
                sq0 = 512 * sqt
                if True:
                    hh = 2 * pr + h
                    for k in range(4):
                        ot = pps.tile([128, 65], F32, tag="pj", name="ot")
                        nc.tensor.transpose(
                            ot[:], ov_acc[h][:, 128 * k:128 * (k + 1)], ident[:])
                        rc = osb.tile([128, 1], F32, tag="rc", name="rc")
                        nc.vector.reciprocal(rc[:], ot[:, 64:65])
                        ob = osb.tile([128, HD], F32, tag="ob", name="ob")
                        nc.vector.tensor_scalar(
                            out=ob[:], in0=ot[:, 0:HD], scalar1=rc[:],
                            scalar2=None, op0=ALU.mult)
                        r0 = sq0 + 128 * k
                        nc.sync.dma_start(
                            out[r0:r0 + 128, HD * hh:HD * (hh + 1)], ob[:])

            def attention_drain(ov_acc, pr, sqt):
                """Transpose + normalize + store the (pr, sqt) block."""
                for h in range(2):
                    drain_head(ov_acc, pr, sqt, h)

            def new_block(pr, sqt):
                a = osb.tile([65, 512], F32, tag="ov", bufs=12, name=f"ova{pr}{sqt}")
                b = osb.tile([65, 512], F32, tag="ov", bufs=12, name=f"ovb{pr}{sqt}")
                return (a, b)

            import contextlib

            def body_scope():
                if loop_n > 1:
                    return tc.For_i(0, loop_n, 1)
                return contextlib.nullcontext()

            for _ in range(repeat):
              with body_scope():
                # persistent per-iteration products
                QT_t = qkv.tile([128, 2, S], F32R, tag="QT", name="QT_t")
                KT_t = qkv.tile([128, 4, S], F32R, tag="KT", name="KT_t")

                V_t = qkv.tile([128, SKC, EV], F32R, tag="V", name="V_t")

                ov = {}

                def pieces(*keys):
                    for (pr, sqt, si) in keys:
                        if (pr, sqt) not in ov:
                            ov[(pr, sqt)] = new_block(pr, sqt)
                        attention_piece(ov[(pr, sqt)], pr, sqt, si)

                # si=0 data first, then attention pieces chunk-chase the
                # remaining K/V (+Q) loads; drains are deferred into later
                # blocks' ACT-busy windows.
                xk = load_x(xkT, 0)
                # zero the unused half of each head's K^T (finite * 0) so QK
                # can run at K=128 with the full Q^T pair tile as rhs
                for _h in range(4):
                    _lo, _hi = (64, 128) if _h % 2 == 0 else (0, 64)
                    nc.vector.tensor_scalar(
                        out=KT_t[_lo:_hi, _h, :], in0=xk[_lo:_hi, 0:4, :],
                        scalar1=0.0, scalar2=None, op0=ALU.mult)
                project_qk(xk, wk_t, bk_t, KT_t, 0, kt=True)
                wq_t, bq_t = load_wq()
                xq = load_x(xqT, 0)
                project_qk(xq, wq_t, bq_t, QT_t, 0)
                wv_t, bv_t = load_wv()
                xv = load_x(xvT, 0)
                project_v(xv, wv_t, bv_t, 0)
                pieces((0, 0, 0), (1, 0, 0))

                xk = load_x(xkT, 1)
                project_qk(xk, wk_t, bk_t, KT_t, 1, kt=True)
                xv = load_x(xvT, 1)
                project_v(xv, wv_t, bv_t, 1)
                pieces((0, 0, 1), (1, 0, 1))
                xq = load_x(xqT, 1)
                project_qk(xq, wq_t, bq_t, QT_t, 1)
                pieces((0, 1, 0), (1, 1, 0))

                xk = load_x(xkT, 2)
                project_qk(xk, wk_t, bk_t, KT_t, 2, kt=True)
                xv = load_x(xvT, 2)
                project_v(xv, wv_t, bv_t, 2)
                pieces((0, 0, 2), (1, 0, 2), (0, 1, 1), (1, 1, 1))

                xk = load_x(xkT, 3)
                project_qk(xk, wk_t, bk_t, KT_t, 3, kt=True)
                xv = load_x(xvT, 3)
                project_v(xv, wv_t, bv_t, 3)
                pieces((0, 0, 3), (1, 0, 3))        # sqt0 complete

                xq = load_x(xqT, 2)
                project_qk(xq, wq_t, bq_t, QT_t, 2)
                pieces((0, 1, 2), (1, 1, 2))
                attention_drain(ov[(0, 0)], 0, 0)
                pieces((0, 1, 3), (1, 1, 3))        # sqt1 complete
                attention_drain(ov[(1, 0)], 1, 0)

                xq = load_x(xqT, 3)
                project_qk(xq, wq_t, bq_t, QT_t, 3)
                pieces((0, 2, 0), (1, 2, 0))
                attention_drain(ov[(0, 1)], 0, 1)
                pieces((0, 2, 1), (1, 2, 1))
                attention_drain(ov[(1, 1)], 1, 1)
                pieces((0, 2, 2), (1, 2, 2), (0, 2, 3), (1, 2, 3))

                pieces((0, 3, 0), (1, 3, 0))
                attention_drain(ov[(0, 2)], 0, 2)
                pieces((0, 3, 1), (1, 3, 1))
                attention_drain(ov[(1, 2)], 1, 2)
                pieces((0, 3, 2), (1, 3, 2), (0, 3, 3), (1, 3, 3))
                attention_drain(ov[(0, 3)], 0, 3)
                attention_drain(ov[(1, 3)], 1, 3)
    nc.compile()
    return nc


_NC_CACHE = {}


def _get_nc(repeat: int = 1, loop_n: int = 1):
    key = (repeat, loop_n)
    if key not in _NC_CACHE:
        _NC_CACHE[key] = build_kernel(repeat, loop_n)
    return _NC_CACHE[key]


def _shard_inputs(q, k, v, Wq, bq, Wk, bk, Wv, bv):
    """Build the 8 per-core input maps (host-side marshaling)."""
    xT = {}
    for b in range(B):
        xT[("q", b)] = np.ascontiguousarray(np.asarray(q)[b].T)
        xT[("k", b)] = np.ascontiguousarray(np.asarray(k)[b].T)
        xT[("v", b)] = np.ascontiguousarray(np.asarray(v)[b].T)
    Wq, Wk, Wv = (np.asarray(a, np.float32) for a in (Wq, Wk, Wv))
    bq, bk, bv = (np.asarray(a, np.float32) for a in (bq, bk, bv))
    in_maps = []
    for c in range(NCORES):
        b, g = divmod(c, HPC)
        sl = slice(E * g, E * (g + 1))
        wv_p = np.zeros((D, EV), np.float32)
        bv_p = np.zeros((128, EV), np.float32)
        for h in range(HPC):
            wv_p[:, 65 * h:65 * h + HD] = Wv[:, E * g + HD * h:E * g + HD * (h + 1)]
            bv_p[:, 65 * h:65 * h + HD] = bv[E * g + HD * h:E * g + HD * (h + 1)]
            bv_p[:, 65 * h + HD] = 1.0
        in_maps.append({
            "xqT": xT[("q", b)], "xkT": xT[("k", b)], "xvT": xT[("v", b)],
            "wq": np.ascontiguousarray(Wq[:, sl]),
            "wk": np.ascontiguousarray(Wk[:, sl]),
            "wv": wv_p,
            "bq": np.ascontiguousarray(bq[sl].reshape(2, 128).T),
            "bk": np.ascontiguousarray(bk[sl].reshape(2, 128).T),
            "bv": bv_p,
        })
    return in_maps


def kernel(q, k, v, Wq, bq, Wk, bk, Wv, bv):
    nc = _get_nc()
    in_maps = _shard_inputs(q, k, v, Wq, bq, Wk, bk, Wv, bv)
    res = run_bass_kernel_spmd(nc, in_maps, core_ids=list(range(NCORES)))
    outp = np.empty((B, S, D), np.float32)
    for c in range(NCORES):
        b, g = divmod(c, HPC)
        outp[b, :, E * g:E * (g + 1)] = res.results[c]["out"]
    return outp


# revision 20
# speedup vs baseline: 7.4602x; 1.9923x over previous
"""Bass/Tile TRN2 kernel for nn_AttentionLayer (B=2, S=2048, D=1024, H=16).

Sharding: 8 cores = 2 (batch) x 4 (head groups of 4 heads each).
Each core computes Q/K/V projections for its 256 output columns and
full attention for its 4 heads; host concatenates the per-core
[S, 256] output slices.

Device-side layout choices:
  - Host pre-transposes q/k/v to x^T [D, S] so projections contract D on
    the partition dim with no on-device transposes.
  - Q^T, K^T produced head-transposed [e, s]; V produced natural [s, e]
    with a fused all-ones column per head (denominator rides the PV
    matmul as output row 64).
  - scores^T = K Q^T per head; softmax exp on ScalarE from PSUM (scale
    1/8 fused); no max-subtraction (scores are O(10), fp32 exp safe).
  - PV: out^T[h d+1, sq] = V'^T E accumulated over sk chunks in PSUM.
  - PE transpose of out^T -> out, then normalize by the ones-row sum.
  - All matmuls in float32r (TF32-like, 1 cycle/row at N>=256).
"""

import os
import sys

sys.path.insert(0, "/opt/trn_rl_repo")

VARIANT = os.environ.get("KVARIANT", "base")

import numpy as np

import concourse.bacc as bacc
import concourse.mybir as mybir
from concourse.masks import make_identity
from concourse.tile import TileContext
from concourse.bass_utils import run_bass_kernel_spmd

F32 = mybir.dt.float32
F32R = mybir.dt.float32r
AF = mybir.ActivationFunctionType
ALU = mybir.AluOpType

B, S, D, H = 2, 2048, 1024, 16
HD = D // H            # 64
NCORES = 8
HPC = 4                # heads per core
E = HPC * HD           # 256 output cols per core
EV = HPC * (HD + 1)    # 260: V' with ones column per head
DCH = D // 128         # 8 d chunks
ST = S // 512          # 4 s tiles (projections)
SQT = S // 1024        # 2 sq tiles (attention)
SKC = S // 128         # 16 sk chunks
SCALE = 1.0 / np.sqrt(HD)


def build_kernel(repeat: int = 1, loop_n: int = 1):
    nc = bacc.Bacc()
    xqT = nc.dram_tensor("xqT", [D, S], F32R, kind="ExternalInput")
    xkT = nc.dram_tensor("xkT", [D, S], F32R, kind="ExternalInput")
    xvT = nc.dram_tensor("xvT", [D, S], F32R, kind="ExternalInput")
    wq = nc.dram_tensor("wq", [D, E], F32R, kind="ExternalInput")
    wk = nc.dram_tensor("wk", [D, E], F32R, kind="ExternalInput")
    wv = nc.dram_tensor("wv", [D, EV], F32R, kind="ExternalInput")
    bq = nc.dram_tensor("bq", [128, 2], F32, kind="ExternalInput")
    bk = nc.dram_tensor("bk", [128, 2], F32, kind="ExternalInput")
    bv = nc.dram_tensor("bv", [128, EV], F32, kind="ExternalInput")
    out = nc.dram_tensor("out", [S, E], F32, kind="ExternalOutput")

    with TileContext(nc) as tc:
        with tc.tile_pool(name="wsb", bufs=1) as wsb, \
             tc.tile_pool(name="xsb", bufs=3) as xsb, \
             tc.tile_pool(name="qkv", bufs=1) as qkv, \
             tc.tile_pool(name="esb", bufs=4) as esb, \
             tc.tile_pool(name="osb", bufs=4) as osb, \
             tc.tile_pool(name="pps", bufs=1, space="PSUM") as pps, \
             tc.tile_pool(name="stp", bufs=2, space="PSUM") as stp, \
             tc.tile_pool(name="pvp", bufs=3, space="PSUM") as pvp:

            # ---- constants / weights ----
            # load order matters at startup: wk then xk0 feed the first
            # matmuls; everything else trails behind them.
            wk_t = wsb.tile([128, DCH, E], F32R)
            nc.sync.dma_start(wk_t[:], wk.rearrange("(c p) e -> p c e", p=128))
            bk_t = wsb.tile([128, 2], F32)
            nc.sync.dma_start(bk_t[:], bk[:])
            ident = wsb.tile([65, 65], F32)
            make_identity(nc, ident[:])
            # touch Exp early so the ACT table load happens during projections
            warm = wsb.tile([128, 1], F32)
            nc.scalar.activation(warm[:], bk_t[:, 0:1], AF.Exp)

            def load_wq():
                wq_t = wsb.tile([128, DCH, E], F32R, name="wq_t")
                nc.sync.dma_start(wq_t[:], wq.rearrange("(c p) e -> p c e", p=128))
                bq_t = wsb.tile([128, 2], F32, name="bq_t")
                nc.sync.dma_start(bq_t[:], bq[:])
                return wq_t, bq_t

            def load_wv():
                wv_t = wsb.tile([128, DCH, EV], F32R, name="wv_t")
                nc.sync.dma_start(wv_t[:], wv.rearrange("(c p) e -> p c e", p=128))
                bv_t = wsb.tile([128, EV], F32, name="bv_t")
                nc.sync.dma_start(bv_t[:], bv[:])
                return wv_t, bv_t

            def load_x(src, si):
                sl = slice(512 * si, 512 * (si + 1))
                x_t = xsb.tile([128, DCH, 512], F32R, tag="x", name=f"x_{si}")
                nc.sync.dma_start(
                    x_t[:], src[:, sl].rearrange("(c p) s -> p c s", p=128))
                return x_t

            def project_qk(x_t, w_t, b_t, o_t, si, kt=False):
                sl = slice(512 * si, 512 * (si + 1))
                for et in range(2):
                    ps = pps.tile([128, 512], F32, tag="pj", name="ps_qk")
                    for c in range(DCH):
                        nc.tensor.matmul(
                            ps[:], w_t[:, c, 128 * et:128 * (et + 1)],
                            x_t[:, c], start=(c == 0), stop=(c == DCH - 1))
                    if kt:
                        nc.vector.tensor_scalar(
                            out=o_t[0:64, 2 * et, sl], in0=ps[0:64, :],
                            scalar1=b_t[0:64, et:et + 1], scalar2=None,
                            op0=ALU.add)
                        nc.vector.tensor_scalar(
                            out=o_t[64:128, 2 * et + 1, sl], in0=ps[64:128, :],
                            scalar1=b_t[64:128, et:et + 1], scalar2=None,
                            op0=ALU.add)
                    else:
                        nc.vector.tensor_scalar(
                            out=o_t[:, et, sl], in0=ps[:],
                            scalar1=b_t[:, et:et + 1], scalar2=None,
                            op0=ALU.add)

            def project_v(x_t, wv_t, bv_t, si):
                for k in range(4):
                    psv = pps.tile([128, EV], F32, tag="pj", name="ps_v")
                    for c in range(DCH):
                        nc.tensor.matmul(
                            psv[:], x_t[:, c, 128 * k:128 * (k + 1)],
                            wv_t[:, c], start=(c == 0), stop=(c == DCH - 1))
                    nc.vector.tensor_tensor(
                        out=V_t[:, 4 * si + k, :], in0=psv[:], in1=bv_t[:],
                        op=ALU.add)

            econst = {}

            def get_econst():
                if "t" not in econst:
                    e0 = esb.tile([128, 1024], F32R, tag="ec", bufs=1, name="e0")
                    nc.vector.tensor_scalar(
                        out=e0[:], in0=wk_t[:, 0:4, 0:256], scalar1=0.0,
                        scalar2=1.0, op0=ALU.mult, op1=ALU.add)
                    econst["t"] = e0
                return econst["t"]

            def attention_piece(ov_acc, pr, sqt, si):
                """Chunks 4si..4si+3 of the (pr, sqt) block.

                PV partials land in a transient PSUM tile per chunk pair and
                are accumulated into ov_acc[h] (SBUF) on the DVE, so only one
                pv PSUM slot is held at a time and many blocks can be in
                flight chunk-chasing the K/V loads.
                """
                sq0 = 512 * sqt
                pv_a = pvp.tile([65, 512], F32, tag="pv", name="pv_a")
                pv_b = pvp.tile([65, 512], F32, tag="pv", name="pv_b")
                pvs = (pv_a, pv_b)
                for cpl in range(2):           # chunk pairs within the piece
                    sts = []
                    if VARIANT != "noqk":
                        for h in range(2):
                            st = stp.tile([128, 1024], F32, tag="st", name=f"st{h}")
                            sts.append(st)
                        for q in range(2):
                            for h in range(2):
                                hh = 2 * pr + h
                                ck = 4 * si + 2 * cpl + q
                                nc.tensor.matmul(
                                    sts[h][:, 512 * q:512 * (q + 1)],
                                    KT_t[:, hh, 128 * ck:128 * (ck + 1)],
                                    QT_t[:, pr, sq0:sq0 + 512],
                                    start=True, stop=True)
                    for h in range(2):
                        hh = 2 * pr + h
                        if VARIANT == "noexp":
                            e_t = get_econst()
                        elif VARIANT == "noqk":
                            e_t = esb.tile([128, 1024], F32R, name="e_t")
                            nc.scalar.activation(e_t[:], QT_t[:, 0, 0:1024],
                                                 AF.Exp, scale=float(SCALE))
                        else:
                            e_t = esb.tile([128, 1024], F32R, name="e_t")
                            nc.scalar.activation(e_t[:], sts[h][:], AF.Exp,
                                                 scale=float(SCALE))
                        if VARIANT != "nopv":
                            for q in range(2):
                                ck = 4 * si + 2 * cpl + q
                                nc.tensor.matmul(
                                    pvs[h][:],
                                    V_t[:, ck, 65 * hh:65 * hh + 65],
                                    e_t[:, 512 * q:512 * (q + 1)],
                                    start=(cpl == 0 and q == 0),
                                    stop=(cpl == 1 and q == 1))
                for h in range(2):
                    if VARIANT == "nopv":
                        if si == 0:
                            nc.vector.tensor_scalar(
                                out=ov_acc[h][:], in0=KT_t[0:65, 0, 0:512],
                                scalar1=0.0, scalar2=1.0, op0=ALU.mult,
                                op1=ALU.add)
                    elif si == 0:
                        nc.vector.tensor_copy(ov_acc[h][:], pvs[h][:])
                    else:
                        nc.vector.tensor_tensor(
                            out=ov_acc[h][:], in0=ov_acc[h][:], in1=pvs[h][:],
                            op=ALU.add)

            def drain_head(ov_acc, pr, sqt, h):
                sq0 = 512 * sqt
                if True:
                    hh = 2 * pr + h
                    for k in range(4):
                        ot = pps.tile([128, 65], F32, tag="pj", name="ot")
                        nc.tensor.transpose(
                            ot[:], ov_acc[h][:, 128 * k:128 * (k + 1)], ident[:])
                        rc = osb.tile([128, 1], F32, tag="rc", name="rc")
                        nc.vector.reciprocal(rc[:], ot[:, 64:65])
                        ob = osb.tile([128, HD], F32, tag="ob", name="ob")
                        nc.vector.tensor_scalar(
                            out=ob[:], in0=ot[:, 0:HD], scalar1=rc[:],
                            scalar2=None, op0=ALU.mult)
                        r0 = sq0 + 128 * k
                        nc.sync.dma_start(
                            out[r0:r0 + 128, HD * hh:HD * (hh + 1)], ob[:])

            def attention_drain(ov_acc, pr, sqt):
                """Transpose + normalize + store the (pr, sqt) block."""
                for h in range(2):
                    drain_head(ov_acc, pr, sqt, h)

            def new_block(pr, sqt):
                a = osb.tile([65, 512], F32, tag="ov", bufs=12, name=f"ova{pr}{sqt}")
                b = osb.tile([65, 512], F32, tag="ov", bufs=12, name=f"ovb{pr}{sqt}")
                return (a, b)

            import contextlib

            def body_scope():
                if loop_n > 1:
                    return tc.For_i(0, loop_n, 1)
                return contextlib.nullcontext()

            for _ in range(repeat):
              with body_scope():
                # persistent per-iteration products
                QT_t = qkv.tile([128, 2, S], F32R, tag="QT", name="QT_t")
                KT_t = qkv.tile([128, 4, S], F32R, tag="KT", name="KT_t")

                V_t = qkv.tile([128, SKC, EV], F32R, tag="V", name="V_t")

                ov = {}

                def pieces(*keys):
                    for (pr, sqt, si) in keys:
                        if (pr, sqt) not in ov:
                            ov[(pr, sqt)] = new_block(pr, sqt)
                        attention_piece(ov[(pr, sqt)], pr, sqt, si)

                # si=0 data first, then attention pieces chunk-chase the
                # remaining K/V (+Q) loads; drains are deferred into later
                # blocks' ACT-busy windows.
                xk = load_x(xkT, 0)
                # zero the unused half of each head's K^T (finite * 0) so QK
                # can run at K=128 with the full Q^T pair tile as rhs
                for _h in range(4):
                    _lo, _hi = (64, 128) if _h % 2 == 0 else (0, 64)
                    nc.vector.tensor_scalar(
                        out=KT_t[_lo:_hi, _h, :], in0=xk[_lo:_hi, 0:4, :],
                        scalar1=0.0, scalar2=None, op0=ALU.mult)
                project_qk(xk, wk_t, bk_t, KT_t, 0, kt=True)
                wq_t, bq_t = load_wq()
                xq = load_x(xqT, 0)
                project_qk(xq, wq_t, bq_t, QT_t, 0)
                wv_t, bv_t = load_wv()
                xv = load_x(xvT, 0)
                project_v(xv, wv_t, bv_t, 0)
                pieces((0, 0, 0), (1, 0, 0))

                xk = load_x(xkT, 1)
                project_qk(xk, wk_t, bk_t, KT_t, 1, kt=True)
                xv = load_x(xvT, 1)
                project_v(xv, wv_t, bv_t, 1)
                pieces((0, 0, 1), (1, 0, 1))
                xq = load_x(xqT, 1)
                project_qk(xq, wq_t, bq_t, QT_t, 1)
                pieces((0, 1, 0), (1, 1, 0))

                xk = load_x(xkT, 2)
                project_qk(xk, wk_t, bk_t, KT_t, 2, kt=True)
                xv = load_x(xvT, 2)
                project_v(xv, wv_t, bv_t, 2)
                pieces((0, 0, 2), (1, 0, 2), (0, 1, 1), (1, 1, 1))

                xk = load_x(xkT, 3)
                project_qk(xk, wk_t, bk_t, KT_t, 3, kt=True)
                xv = load_x(xvT, 3)
                project_v(xv, wv_t, bv_t, 3)
                pieces((0, 0, 3), (1, 0, 3))        # sqt0 complete

                xq = load_x(xqT, 2)
                project_qk(xq, wq_t, bq_t, QT_t, 2)
                pieces((0, 1, 2), (1, 1, 2))
                attention_drain(ov[(0, 0)], 0, 0)
                pieces((0, 1, 3), (1, 1, 3))        # sqt1 complete
                attention_drain(ov[(1, 0)], 1, 0)

                xq = load_x(xqT, 3)
                project_qk(xq, wq_t, bq_t, QT_t, 3)
                pieces((0, 2, 0), (1, 2, 0))
                attention_drain(ov[(0, 1)], 0, 1)
                pieces((0, 2, 1), (1, 2, 1))
                attention_drain(ov[(1, 1)], 1, 1)
                pieces((0, 2, 2), (1, 2, 2), (0, 2, 3), (1, 2, 3))

                pieces((0, 3, 0), (1, 3, 0))
                attention_drain(ov[(0, 2)], 0, 2)
                pieces((0, 3, 1), (1, 3, 1))
                attention_drain(ov[(1, 2)], 1, 2)
                pieces((0, 3, 2), (1, 3, 2), (0, 3, 3), (1, 3, 3))
                attention_drain(ov[(0, 3)], 0, 3)
                attention_drain(ov[(1, 3)], 1, 3)
    nc.compile()
    return nc


_NC_CACHE = {}


def _get_nc(repeat: int = 1, loop_n: int = 1):
    key = (repeat, loop_n)
    if key not in _NC_CACHE:
        _NC_CACHE[key] = build_kernel(repeat, loop_n)
    return _NC_CACHE[key]


def _shard_inputs(q, k, v, Wq, bq, Wk, bk, Wv, bv):
    """Build the 8 per-core input maps (host-side marshaling)."""
    xT = {}
    for b in range(B):
        xT[("q", b)] = np.ascontiguousarray(np.asarray(q)[b].T)
        xT[("k", b)] = np.ascontiguousarray(np.asarray(k)[b].T)
        xT[("v", b)] = np.ascontiguousarray(np.asarray(v)[b].T)
    Wq, Wk, Wv = (np.asarray(a, np.float32) for a in (Wq, Wk, Wv))
    bq, bk, bv = (np.asarray(a, np.float32) for a in (bq, bk, bv))
    in_maps = []
    for c in range(NCORES):
        b, g = divmod(c, HPC)
        sl = slice(E * g, E * (g + 1))
        wv_p = np.zeros((D, EV), np.float32)
        bv_p = np.zeros((128, EV), np.float32)
        for h in range(HPC):
            wv_p[:, 65 * h:65 * h + HD] = Wv[:, E * g + HD * h:E * g + HD * (h + 1)]
            bv_p[:, 65 * h:65 * h + HD] = bv[E * g + HD * h:E * g + HD * (h + 1)]
            bv_p[:, 65 * h + HD] = 1.0
        in_maps.append({
            "xqT": xT[("q", b)], "xkT": xT[("k", b)], "xvT": xT[("v", b)],
            "wq": np.ascontiguousarray(Wq[:, sl]),
            "wk": np.ascontiguousarray(Wk[:, sl]),
            "wv": wv_p,
            "bq": np.ascontiguousarray(bq[sl].reshape(2, 128).T),
            "bk": np.ascontiguousarray(bk[sl].reshape(2, 128).T),
            "bv": bv_p,
        })
    return in_maps


def kernel(q, k, v, Wq, bq, Wk, bk, Wv, bv):
    nc = _get_nc()
    in_maps = _shard_inputs(q, k, v, Wq, bq, Wk, bk, Wv, bv)
    res = run_bass_kernel_spmd(nc, in_maps, core_ids=list(range(NCORES)))
    outp = np.empty((B, S, D), np.float32)
    for c in range(NCORES):
        b, g = divmod(c, HPC)
        outp[b, :, E * g:E * (g + 1)] = res.results[c]["out"]
    return outp
